# revision 29
# baseline (speedup 1.0000x reference)
"""Trainium2 Bass kernel for ColorToneMapper MLP.

color = tanh(W3^T relu(W2^T relu(W1^T relu(W0^T safelog(radience)))))

Fast path (used when all biases are zero, as in this problem's inputs):
the MLP input is the scalar t = safelog(r) and every bias is zero, so
each ReLU's sign pattern depends only on sign(t) — for t < 0 (r < 1,
always true for these inputs) the mask is a fixed per-weight constant.
The whole network therefore collapses to

    color = tanh(c * t),   c = W3^T M2 W2^T M1 W1^T M0 W0

with M* the constant sign masks, and c computed on the host from the
weights.  The device kernel is then a pure memory-bound elementwise map
over the 2M pixels, data-parallel across 8 cores.

Primary device pipeline (poly): the host sends 1/max(r, eps) in bf16
(halves the input DMA) and a degree-5 odd minimax fit of tanh(|c|m) on
m = ln(1/r) in [0, m_max], factored for scalar_tensor_tensor form:

    m = Ln(recip)                      # ACT, 1 pass, bf16 -> fp16
    w = (m * s1) * m                   # DVE STT
    y = (w + s2) * w                   # DVE STT
    o = (y + s3) * m                   # DVE STT  == s1^2 m^5 + s1 s2 m^3 + s3 m
    out fp16 -> host upcasts to f32    # halves the output DMA

(fit absmax error ~1e-3, fp16-rounding-everywhere end-to-end ~2.9e-3
vs the 2e-2 gate).  Only Ln is needed on ACT, so a single activation
table load suffices and no table switch lands on the critical path.

Fallback 1 (exp), if the poly fit is ill-conditioned for the actual c:
f32 path computing v = exp(2c ln r); color = 1 - 2/(v+1) via DVE
reciprocal_approx_fast, with an explicit combined ln+exp table load.

Fallback 2 (nonzero biases): the original per-pixel MLP kernel.
"""

import numpy as np

N_TOTAL = 2097152
N_CORES = 8
N_CORE = N_TOTAL // N_CORES  # 262144
P = 128                      # SBUF partitions
CH = 512                     # chunk width = one PSUM bank of fp32 (MLP path)
SLAB = 8 * CH                # pixels per slab (MLP path)
EPS = 1e-8
N_TILES = 8                  # exp-path tiles per core
N_TILES_POLY = 4             # poly-path tiles per core

_BUILT_POLY = None  # cached Bass module (poly fast path)
_BUILT_FAST = None  # cached Bass module (exp fast path)
_BUILT = None       # cached Bass module (MLP fallback path)


# ---------------------------------------------------------------------------
# fast path: color = tanh(c * safelog(r))
# ---------------------------------------------------------------------------

def _collapse_scalar(W0, W1, W2, W3, sign):
    """Fold the zero-bias MLP into a single scalar for inputs with
    sign(safelog(r)) == sign.  relu(t*v) = t * (v masked to sign*v > 0)."""
    m0 = np.where(sign * W0 > 0, W0, 0.0)
    v1 = m0 @ W1
    m1 = np.where(sign * v1 > 0, v1, 0.0)
    v2 = m1 @ W2
    m2 = np.where(sign * v2 > 0, v2, 0.0)
    return float(m2 @ W3)


def _build_poly(a, b, s, n_core=N_CORE):
    """Device program for out = (Square(a*t + b) + s) * t with
    t = Ln(r_clamped) — a cubic fit of tanh(c*t) on t in [t_min, 0].
    Coefficients are baked as immediates (module rebuilt per weight set;
    compile is ~2 s).  Three passes per element:

      ACT: t = Ln(rin)              (bf16 in -> fp16)
      ACT: y = Square(a*t + b)      (free affine inside the activation)
      DVE: out = (y + s) * t        (one scalar_tensor_tensor)

    Ln and Square share the natural_log table set: one load, no switch.
    """
    from concourse import bacc
    import concourse.tile as tile
    from concourse import mybir
    from contextlib import ExitStack

    f16 = mybir.dt.float16
    bf16 = mybir.dt.bfloat16
    A = mybir.ActivationFunctionType
    ALU = mybir.AluOpType

    p = P
    f = n_core // p        # 2048 elements per partition
    # uneven tiles: a small last tile shortens the end-of-kernel
    # dependency chain (its Ln/Square/STT/out-DMA tail)
    splits = [0, 640, 1792, 2048]

    nc = bacc.Bacc("TRN2", target_bir_lowering=False, debug=False)

    rin_d = nc.dram_tensor("rin", [n_core], bf16, kind="ExternalInput")
    out_d = nc.dram_tensor("color16", [n_core], f16, kind="ExternalOutput")
    rin2d = rin_d.ap().rearrange("(p f) -> p f", p=p)
    out2d = out_d.ap().rearrange("(p f) -> p f", p=p)

    combined_id = None
    try:
        from concourse.hw_specs import get_activation_tables
        names = list(get_activation_tables(nc.m.arch).keys())
        combined_id = names.index("natural_log")
    except Exception:
        combined_id = None

    f32 = mybir.dt.float32
    with tile.TileContext(nc) as tc, ExitStack() as ctx:
        consts = ctx.enter_context(tc.tile_pool(name="consts", bufs=1))
        iop = ctx.enter_context(tc.tile_pool(name="iop", bufs=4))
        tp = ctx.enter_context(tc.tile_pool(name="tp", bufs=4))
        wp = ctx.enter_context(tc.tile_pool(name="wp", bufs=4))
        oop = ctx.enter_context(tc.tile_pool(name="oop", bufs=4))

        # float activation bias requires a registered const AP; memset our
        # own [128,1] bias tile instead
        bt = consts.tile([p, 1], f32)
        nc.gpsimd.memset(bt[:], float(b))

        if combined_id is not None:
            nc.scalar.add_instruction(
                mybir.InstLoadActFuncSet(
                    name=nc.get_next_instruction_name(),
                    act_func_set_id=combined_id,
                )
            )

        for i in range(len(splits) - 1):
            lo, hi = splits[i], splits[i + 1]
            ft = hi - lo
            v = iop.tile([p, ft], bf16, tag="v")
            nc.sync.dma_start(out=v[:], in_=rin2d[:, lo:hi])
            t = tp.tile([p, ft], f16, tag="t")
            nc.scalar.activation(out=t[:], in_=v[:], func=A.Ln)
            w = wp.tile([p, ft], f16, tag="w")
            nc.scalar.activation(out=w[:], in_=t[:], func=A.Square,
                                 scale=float(a), bias=bt[:])
            o = oop.tile([p, ft], f16, tag="o")
            nc.vector.scalar_tensor_tensor(
                out=o[:], in0=w[:], scalar=float(s), in1=t[:],
                op0=ALU.add, op1=ALU.mult,
            )
            nc.sync.dma_start(out=out2d[:, lo:hi], in_=o[:])

    nc.finalize()
    return nc


def _fit_poly(c, t_min):
    """Weighted-lstsq (approx minimax) cubic fit of tanh(c*t) on
    [t_min, 0], factored as (Square(a*t+b)+s)*t = a^2 t^3 + 2ab t^2 +
    (b^2+s) t.  Returns (a, b, s, fit_maxerr, y_max) or None if the
    t^3 coefficient is not positive (the Square form needs a^2 = alpha)."""
    t = np.linspace(min(t_min, -1e-3), 0.0, 20001)
    y = np.tanh(c * t)
    A = np.stack([t ** 3, t ** 2, t], axis=1)
    w = np.ones_like(t)
    coef = np.zeros(3)
    for _ in range(25):
        coef, *_ = np.linalg.lstsq(A * w[:, None], y * w, rcond=None)
        err = A @ coef - y
        w = (w * (1 + np.abs(err) / max(np.abs(err).max(), 1e-30))) ** 0.8
        w /= w.max()
    al, be, ga = coef
    maxerr = float(np.abs(A @ coef - y).max())
    if not (al > 1e-30):
        return None
    a = float(np.sqrt(al))
    b = float(be / (2 * a))
    return a, b, float(ga - b * b), maxerr, float(np.abs(y).max())


def _build_fast(n_core=N_CORE, n_tiles=N_TILES):
    from concourse import bacc
    import concourse.tile as tile
    from concourse import mybir
    from contextlib import ExitStack

    f32 = mybir.dt.float32
    A = mybir.ActivationFunctionType
    ALU = mybir.AluOpType

    p = P
    f = n_core // p        # 2048 elements per partition
    ft = f // n_tiles

    nc = bacc.Bacc("TRN2", target_bir_lowering=False, debug=False)

    rad_d = nc.dram_tensor("radience", [n_core], f32, kind="ExternalInput")
    c_d = nc.dram_tensor("two_c", [1], f32, kind="ExternalInput")
    out_d = nc.dram_tensor("color", [n_core], f32, kind="ExternalOutput")
    rad2d = rad_d.ap().rearrange("(p f) -> p f", p=p)
    out2d = out_d.ap().rearrange("(p f) -> p f", p=p)

    # ln and exp never share a compiler-chosen table set; preload the one
    # set that holds both so no ACT table switch lands on the critical path
    combined_id = None
    try:
        from concourse.hw_specs import get_activation_tables
        names = list(get_activation_tables(nc.m.arch).keys())
        combined_id = names.index("natural_log_exp_and_others")
    except Exception:
        combined_id = None  # compiler inserts per-function loads instead

    with tile.TileContext(nc) as tc, ExitStack() as ctx:
        consts = ctx.enter_context(tc.tile_pool(name="consts", bufs=1))
        iop = ctx.enter_context(tc.tile_pool(name="iop", bufs=4))
        oop = ctx.enter_context(tc.tile_pool(name="oop", bufs=4))

        if combined_id is not None:
            nc.scalar.add_instruction(
                mybir.InstLoadActFuncSet(
                    name=nc.get_next_instruction_name(),
                    act_func_set_id=combined_id,
                )
            )

        cs = consts.tile([p, 1], f32)
        nc.sync.dma_start(out=cs[:], in_=c_d.ap().to_broadcast([p, 1]))

        for i in range(n_tiles):
            v = iop.tile([p, ft], f32, tag="v")
            nc.sync.dma_start(out=v[:], in_=rad2d[:, i * ft:(i + 1) * ft])
            nc.scalar.activation(out=v[:], in_=v[:], func=A.Ln)
            # v = exp(2c * ln r) = r^(2c); per-partition scale operand
            nc.scalar.activation(out=v[:], in_=v[:], func=A.Exp, scale=cs[:])
            w = oop.tile([p, ft], f32, tag="w")
            nc.vector.tensor_scalar(
                out=w[:], in0=v[:], scalar1=1.0, scalar2=None, op0=ALU.add
            )
            nc.vector.reciprocal_approx_fast(out=w[:], in_=w[:])
            # tanh = 1 - 2/(v+1)
            nc.vector.tensor_scalar(
                out=w[:], in0=w[:], scalar1=-2.0, scalar2=1.0,
                op0=ALU.mult, op1=ALU.add,
            )
            nc.sync.dma_start(out=out2d[:, i * ft:(i + 1) * ft], in_=w[:])

    nc.finalize()
    return nc


def _kernel_poly(radc, c_neg):
    """Device poly path; returns f32 [N_TOTAL] or None if the fit for
    this c is unusable."""
    global _BUILT_POLY
    from ml_dtypes import bfloat16

    t_min = float(np.log(max(float(radc.min()), EPS)))
    fit = _fit_poly(c_neg, t_min)
    if fit is None:
        return None
    a, b, s, maxerr, y_max = fit
    # the fit must hold well inside the 2e-2-of-absmax gate (fp16
    # rounding adds ~1e-3)
    if maxerr > 0.006 * max(y_max, 0.05):
        return None

    key = (round(a, 12), round(b, 12), round(s, 12))
    if _BUILT_POLY is None or _BUILT_POLY[0] != key:
        _BUILT_POLY = (key, _build_poly(a, b, s))
    nc = _BUILT_POLY[1]

    rin = radc.astype(bfloat16)
    in_maps = []
    for c in range(N_CORES):
        in_maps.append({
            "rin": np.ascontiguousarray(rin[c * N_CORE:(c + 1) * N_CORE]),
        })
    res = _run(nc, in_maps, list(range(N_CORES)))
    out = np.concatenate([res.results[c]["color16"] for c in range(N_CORES)])
    return out.astype(np.float32)


def _kernel_exp(radc, c_neg):
    global _BUILT_FAST
    if _BUILT_FAST is None:
        _BUILT_FAST = _build_fast()
    nc = _BUILT_FAST

    two_c = np.array([2.0 * c_neg], dtype=np.float32)
    in_maps = []
    for c in range(N_CORES):
        in_maps.append({
            "radience": np.ascontiguousarray(radc[c * N_CORE:(c + 1) * N_CORE]),
            "two_c": two_c,
        })
    res = _run(nc, in_maps, list(range(N_CORES)))
    out = np.concatenate([res.results[c]["color"] for c in range(N_CORES)])
    return np.asarray(out, dtype=np.float32)


def _kernel_fast(rad, inputs):
    W0 = np.asarray(inputs["W0"], dtype=np.float64).reshape(-1)
    W1 = np.asarray(inputs["W1"], dtype=np.float64)
    W2 = np.asarray(inputs["W2"], dtype=np.float64)
    W3 = np.asarray(inputs["W3"], dtype=np.float64).reshape(-1)
    c_neg = _collapse_scalar(W0, W1, W2, W3, -1.0)

    radc = np.maximum(rad, np.float32(EPS))  # safelog domain guard

    out = None
    if c_neg < 0:
        out = _kernel_poly(radc, c_neg)
    if out is None:
        out = _kernel_exp(radc, c_neg)
    out = out.reshape(N_TOTAL, 1)

    # pixels with r >= 1 (t >= 0) follow the positive-sign mask chain
    if float(rad.max()) >= 1.0:
        c_pos = _collapse_scalar(W0, W1, W2, W3, 1.0)
        idx = np.nonzero(rad >= 1.0)[0]
        out[idx, 0] = np.tanh(
            c_pos * np.log(np.maximum(rad[idx], EPS))
        ).astype(np.float32)
    return out


# ---------------------------------------------------------------------------
# fallback path: full per-pixel MLP (handles arbitrary biases)
# ---------------------------------------------------------------------------

def _build_bass(n_core=N_CORE, mm_dt_name="float16", finalize=True):
    from concourse import bacc
    import concourse.tile as tile
    from concourse import mybir
    from contextlib import ExitStack

    f32 = mybir.dt.float32
    mm_dt = getattr(mybir.dt, mm_dt_name)
    A = mybir.ActivationFunctionType
    ALU = mybir.AluOpType

    p = P
    f = n_core // p              # free dim per partition
    n_chunks = n_core // CH
    n_slabs = n_core // SLAB
    rows_per_slab = SLAB // f    # rad partition-rows gathered per slab
    assert n_chunks % 8 == 0 and rows_per_slab >= 1

    nc = bacc.Bacc("TRN2", target_bir_lowering=False, debug=False)

    rad_d = nc.dram_tensor("radience", [n_core], f32, kind="ExternalInput")
    out_d = nc.dram_tensor("color", [n_core], f32, kind="ExternalOutput")
    w0_d = nc.dram_tensor("W0", [1, 128], f32, kind="ExternalInput")
    b0_d = nc.dram_tensor("b0", [128], f32, kind="ExternalInput")
    w1_d = nc.dram_tensor("W1", [128, 128], f32, kind="ExternalInput")
    b1_d = nc.dram_tensor("b1", [128], f32, kind="ExternalInput")
    w2_d = nc.dram_tensor("W2", [128, 128], f32, kind="ExternalInput")
    b2_d = nc.dram_tensor("b2", [128], f32, kind="ExternalInput")
    w3_d = nc.dram_tensor("W3", [128, 32], f32, kind="ExternalInput")
    b3_d = nc.dram_tensor("b3", [1], f32, kind="ExternalInput")

    rad2d = rad_d.ap().rearrange("(p f) -> p f", p=p)
    out3d = out_d.ap().rearrange("(g r c) -> g r c", r=4, c=CH)

    with tile.TileContext(nc) as tc, ExitStack() as ctx:
        consts = ctx.enter_context(tc.tile_pool(name="consts", bufs=1))
        radp = ctx.enter_context(tc.tile_pool(name="radp", bufs=1))
        stgp = ctx.enter_context(tc.tile_pool(name="stgp", bufs=4))
        hp = ctx.enter_context(tc.tile_pool(name="hp", bufs=9))
        outp = ctx.enter_context(tc.tile_pool(name="outp", bufs=3))
        psp = ctx.enter_context(tc.tile_pool(name="psp", bufs=4, space="PSUM"))

        # --- constants ---
        # weights land as fp32 then are copy-converted to the matmul dtype
        # (fp32r consumers require producer-side rounding)
        w0f = consts.tile([1, 128], f32)
        nc.sync.dma_start(out=w0f[:], in_=w0_d.ap())
        w1f = consts.tile([128, 128], f32)
        nc.sync.dma_start(out=w1f[:], in_=w1_d.ap())
        w2f = consts.tile([128, 128], f32)
        nc.sync.dma_start(out=w2f[:], in_=w2_d.ap())
        # W3 arrives host-padded to 32 output columns (col 0 real, rest
        # zero) so each column-tiled layer-4 matmul initializes a full
        # 32-partition strip
        w3f = consts.tile([128, 32], f32)
        nc.sync.dma_start(out=w3f[:], in_=w3_d.ap())
        w0 = consts.tile([1, 128], mm_dt)
        nc.vector.tensor_copy(w0[:], w0f[:])
        # W0 replicated onto partitions {0,32,64,96}: layer-1 K=1 matmuls
        # run 4-concurrent on disjoint 32-row strips of the PE array
        w0q = consts.tile([97, 128], mm_dt)
        for _r in range(4):
            nc.sync.dma_start(out=w0q[32 * _r:32 * _r + 1, :], in_=w0[:])
        w1 = consts.tile([128, 128], mm_dt)
        nc.vector.tensor_copy(w1[:], w1f[:])
        w2 = consts.tile([128, 128], mm_dt)
        nc.vector.tensor_copy(w2[:], w2f[:])
        # layer-4 column-tiles, so it must use a 16-bit dtype
        w3 = consts.tile([128, 32], mm_dt)
        nc.vector.tensor_copy(w3[:], w3f[:])
        b0s = consts.tile([128, 1], f32)
        nc.sync.dma_start(out=b0s[:], in_=b0_d.ap().rearrange("(p f) -> p f", f=1))
        b1s = consts.tile([128, 1], f32)
        nc.sync.dma_start(out=b1s[:], in_=b1_d.ap().rearrange("(p f) -> p f", f=1))
        b2s = consts.tile([128, 1], f32)
        nc.sync.dma_start(out=b2s[:], in_=b2_d.ap().rearrange("(p f) -> p f", f=1))
        b3s = consts.tile([128, 1], f32)
        nc.sync.dma_start(out=b3s[:], in_=b3_d.ap().to_broadcast([128, 1]))

        # --- load pixels, safelog ---
        rad = radp.tile([p, f], f32)
        nc.sync.dma_start(out=rad[:], in_=rad2d)
        nc.vector.tensor_scalar(
            out=rad[:], in0=rad[:], scalar1=EPS, scalar2=None, op0=ALU.max
        )
        logr = radp.tile([p, f], mm_dt)
        nc.scalar.activation(out=logr[:], in_=rad[:], func=A.Ln)

        def relu_into(dst, src, bias, use_act):
            if use_act:
                nc.scalar.activation(out=dst, in_=src, func=A.Relu, bias=bias)
            else:
                nc.vector.tensor_scalar(
                    out=dst, in0=src, scalar1=bias, scalar2=0.0,
                    op0=ALU.add, op1=ALU.max,
                )

        prev = None  # software-pipelined layer 4 of slab s-1

        def emit_l4(pv):
            h3p, s_p = pv
            ps4 = psp.tile([128, 2 * CH], f32, tag="ps")
            for j in range(8):
                g, r = j // 4, j % 4
                srcp = h3p[j // 2][:, (j % 2) * CH:(j % 2 + 1) * CH]
                nc.tensor.matmul(
                    out=ps4[32 * r:32 * r + 32, g * CH:(g + 1) * CH],
                    lhsT=w3[:], rhs=srcp,
                    tile_position=(0, 32 * r),
                    skip_group_check=True,
                )
            ot = outp.tile([128, 2 * CH], f32, tag="ot")
            nc.scalar.activation(out=ot[:], in_=ps4[:], func=A.Tanh, bias=b3s[:])
            for g in range(2):
                nc.sync.dma_start(
                    out=out3d[2 * s_p + g, :, :],
                    in_=ot[0:128:32, g * CH:(g + 1) * CH],
                )

        for s in range(n_slabs):
            # gather this slab's log-pixels onto partitions {0,32,64,96}:
            # strip 32r gets chunk r (free 0:CH) and chunk 4+r (free CH:2CH)
            stg = stgp.tile([97, SLAB // 4], mm_dt, tag="stg")
            rs = s * rows_per_slab
            if rows_per_slab == 2:
                # each logr row covers 4 chunks -> one strided DMA per row
                for g in range(2):
                    nc.sync.dma_start(
                        out=stg[0:97:32, g * CH:(g + 1) * CH],
                        in_=logr[rs + g:rs + g + 1, :],
                    )
            else:
                for j in range(8):
                    px = s * SLAB + j * CH
                    row, col = px // f, px % f
                    nc.sync.dma_start(
                        out=stg[32 * (j % 4):32 * (j % 4) + 1,
                                (j // 4) * CH:(j // 4 + 1) * CH],
                        in_=logr[row:row + 1, col:col + CH],
                    )

            # ---- layers 1..3, layer-major so engine FIFOs never
            # head-of-line block: all matmuls of a layer back-to-back
            # (keeps the PE HAM-warm), relus split ACT/DVE per pair ----
            ps1s, h1s, ps2s, h2s, ps3s, h3 = [], [], [], [], [], []
            for q in range(4):
                ps1s.append(psp.tile([128, 2 * CH], f32, tag="ps", name=f"ps1_{s}_{q}"))
            for j in range(8):
                g, r = j // 4, j % 4
                nc.tensor.matmul(
                    out=ps1s[j // 2][:, (j % 2) * CH:(j % 2 + 1) * CH],
                    lhsT=w0q[32 * r:32 * r + 1, :],
                    rhs=stg[32 * r:32 * r + 1, g * CH:(g + 1) * CH],
                    tile_position=(32 * r, 0),
                    skip_group_check=True,
                )
            if prev is not None:
                emit_l4(prev)
            for q in range(4):
                h1 = hp.tile([128, 2 * CH], mm_dt, tag="h")
                relu_into(h1[:], ps1s[q][:], b0s[:], use_act=(q % 2 == 0))
                h1s.append(h1)
            for q in range(4):
                ps2 = psp.tile([128, 2 * CH], f32, tag="ps")
                nc.tensor.matmul(out=ps2[:, 0:CH], lhsT=w1[:],
                                 rhs=h1s[q][:, 0:CH])
                nc.tensor.matmul(out=ps2[:, CH:2 * CH], lhsT=w1[:],
                                 rhs=h1s[q][:, CH:2 * CH])
                ps2s.append(ps2)
            for q in range(4):
                h2 = hp.tile([128, 2 * CH], mm_dt, tag="h")
                relu_into(h2[:], ps2s[q][:], b1s[:], use_act=(q % 2 == 1))
                h2s.append(h2)
            for q in range(4):
                ps3 = psp.tile([128, 2 * CH], f32, tag="ps")
                nc.tensor.matmul(out=ps3[:, 0:CH], lhsT=w2[:],
                                 rhs=h2s[q][:, 0:CH])
                nc.tensor.matmul(out=ps3[:, CH:2 * CH], lhsT=w2[:],
                                 rhs=h2s[q][:, CH:2 * CH])
                ps3s.append(ps3)
            for q in range(4):
                h3q = hp.tile([128, 2 * CH], mm_dt, tag="h3")
                relu_into(h3q[:], ps3s[q][:], b2s[:], use_act=(q % 2 == 0))
                h3.append(h3q)

            prev = (h3, s)

        emit_l4(prev)

    if finalize:
        nc.finalize()
    return nc


def _kernel_mlp(rad, inputs):
    global _BUILT
    weights = {
        k: np.ascontiguousarray(np.asarray(inputs[k], dtype=np.float32))
        for k in ("W0", "b0", "W1", "b1", "W2", "b2", "W3", "b3")
    }
    weights["W3"] = np.ascontiguousarray(
        np.pad(weights["W3"].reshape(128, 1), ((0, 0), (0, 31)))
    )

    if _BUILT is None:
        _BUILT = _build_bass()
    nc = _BUILT

    in_maps = []
    for c in range(N_CORES):
        m = {"radience": np.ascontiguousarray(rad[c * N_CORE:(c + 1) * N_CORE])}
        m.update(weights)
        in_maps.append(m)

    res = _run(nc, in_maps, list(range(N_CORES)))
    out = np.concatenate([res.results[c]["color"] for c in range(N_CORES)])
    return out.reshape(N_TOTAL, 1)


def _run(nc, in_maps, core_ids, **kw):
    from concourse.bass_utils import run_bass_kernel_spmd
    return run_bass_kernel_spmd(nc, in_maps, core_ids, **kw)


def kernel(**inputs):
    rad = np.asarray(inputs["radience"], dtype=np.float32).reshape(-1)
    n = rad.shape[0]
    assert n == N_TOTAL, f"expected {N_TOTAL} pixels, got {n}"

    biases_zero = all(
        np.all(np.asarray(inputs[k]) == 0.0) for k in ("b0", "b1", "b2", "b3")
    )
    if biases_zero:
        return _kernel_fast(rad, inputs)
    return _kernel_mlp(rad, inputs)


if __name__ == "__main__":
    rng = np.random.default_rng(0)
    demo = {
        "radience": rng.random((N_TOTAL, 1), dtype=np.float32),
        "W0": rng.standard_normal((1, 128), dtype=np.float32) * 0.1,
        "b0": np.zeros(128, np.float32),
        "W1": rng.standard_normal((128, 128), dtype=np.float32) * 0.1,
        "b1": np.zeros(128, np.float32),
        "W2": rng.standard_normal((128, 128), dtype=np.float32) * 0.1,
        "b2": np.zeros(128, np.float32),
        "W3": rng.standard_normal((128, 1), dtype=np.float32) * 0.1,
        "b3": np.zeros(1, np.float32),
    }
    out = kernel(**demo)
    print("kernel out:", out.shape, out.dtype, out[:4, 0])


# revision 31
# speedup vs baseline: 1.0203x; 1.0203x over previous
"""Trainium2 Bass kernel for ColorToneMapper MLP.

color = tanh(W3^T relu(W2^T relu(W1^T relu(W0^T safelog(radience)))))

Fast path (used when all biases are zero, as in this problem's inputs):
the MLP input is the scalar t = safelog(r) and every bias is zero, so
each ReLU's sign pattern depends only on sign(t) — for t < 0 (r < 1,
always true for these inputs) the mask is a fixed per-weight constant.
The whole network therefore collapses to

    color = tanh(c * t),   c = W3^T M2 W2^T M1 W1^T M0 W0

with M* the constant sign masks, and c computed on the host from the
weights.  The device kernel is then a pure memory-bound elementwise map
over the 2M pixels, data-parallel across 8 cores.

Primary device pipeline (poly): the host sends max(r, eps) in bf16
(halves the input DMA) and fits a cubic to tanh(c*t) on the actual
t = ln(r) range [t_min, 0], factored into three device passes:

    t = Ln(rin)                        # ACT, bf16 -> fp16
    y = Square(a*t + b)                # ACT (free affine pre-scale/bias)
    o = (y + s) * t                    # DVE scalar_tensor_tensor, fp16
    out fp16 -> host upcasts to f32    # halves the output DMA

== a^2 t^3 + 2ab t^2 + (b^2+s) t; the t^3 coefficient of the fit is
positive for c < 0, matching the Square form's a^2.  Fit absmax error
~2.5e-3, fp16-everywhere end-to-end ~3.2e-3 vs the 2e-2 gate.  Ln and
Square share one ACT table set (natural_log), loaded once via an
explicit InstLoadActFuncSet hidden under the input DMA; coefficients
are baked as immediates (module rebuilt per weight set, ~2 s compile).
Three uneven tiles [512, 1152, 384] keep ACT fed as inputs arrive and
shorten the final Ln->Square->STT->DMA dependency chain.

Fallback 1 (exp), if the poly fit is ill-conditioned for the actual c:
f32 path computing v = exp(2c ln r); color = 1 - 2/(v+1) via DVE
reciprocal_approx_fast, with an explicit combined ln+exp table load.

Fallback 2 (nonzero biases): the original per-pixel MLP kernel.
"""

import numpy as np

N_TOTAL = 2097152
N_CORES = 8
N_CORE = N_TOTAL // N_CORES  # 262144
P = 128                      # SBUF partitions
CH = 512                     # chunk width = one PSUM bank of fp32 (MLP path)
SLAB = 8 * CH                # pixels per slab (MLP path)
EPS = 1e-8
N_TILES = 8                  # exp-path tiles per core
N_TILES_POLY = 4             # poly-path tiles per core

_BUILT_POLY = None  # cached Bass module (poly fast path)
_BUILT_FAST = None  # cached Bass module (exp fast path)
_BUILT = None       # cached Bass module (MLP fallback path)


# ---------------------------------------------------------------------------
# fast path: color = tanh(c * safelog(r))
# ---------------------------------------------------------------------------

def _collapse_scalar(W0, W1, W2, W3, sign):
    """Fold the zero-bias MLP into a single scalar for inputs with
    sign(safelog(r)) == sign.  relu(t*v) = t * (v masked to sign*v > 0)."""
    m0 = np.where(sign * W0 > 0, W0, 0.0)
    v1 = m0 @ W1
    m1 = np.where(sign * v1 > 0, v1, 0.0)
    v2 = m1 @ W2
    m2 = np.where(sign * v2 > 0, v2, 0.0)
    return float(m2 @ W3)


def _build_poly(a, b, s, n_core=N_CORE):
    """Device program for out = (Square(a*t + b) + s) * t with
    t = Ln(r_clamped) — a cubic fit of tanh(c*t) on t in [t_min, 0].
    Coefficients are baked as immediates (module rebuilt per weight set;
    compile is ~2 s).  Three passes per element:

      ACT: t = Ln(rin)              (bf16 in -> fp16)
      ACT: y = Square(a*t + b)      (free affine inside the activation)
      DVE: out = (y + s) * t        (one scalar_tensor_tensor)

    Ln and Square share the natural_log table set: one load, no switch.
    """
    from concourse import bacc
    import concourse.tile as tile
    from concourse import mybir
    from contextlib import ExitStack

    f16 = mybir.dt.float16
    bf16 = mybir.dt.bfloat16
    A = mybir.ActivationFunctionType
    ALU = mybir.AluOpType

    p = P
    f = n_core // p        # 2048 elements per partition
    # uneven tiles: a small last tile shortens the end-of-kernel
    # dependency chain (its Ln/Square/STT/out-DMA tail)
    splits = [0, 512, 1664, 2048]

    nc = bacc.Bacc("TRN2", target_bir_lowering=False, debug=False)

    rin_d = nc.dram_tensor("rin", [n_core], bf16, kind="ExternalInput")
    out_d = nc.dram_tensor("color16", [n_core], f16, kind="ExternalOutput")
    rin2d = rin_d.ap().rearrange("(p f) -> p f", p=p)
    out2d = out_d.ap().rearrange("(p f) -> p f", p=p)

    combined_id = None
    try:
        from concourse.hw_specs import get_activation_tables
        names = list(get_activation_tables(nc.m.arch).keys())
        combined_id = names.index("natural_log")
    except Exception:
        combined_id = None

    f32 = mybir.dt.float32
    with tile.TileContext(nc) as tc, ExitStack() as ctx:
        consts = ctx.enter_context(tc.tile_pool(name="consts", bufs=1))
        iop = ctx.enter_context(tc.tile_pool(name="iop", bufs=4))
        tp = ctx.enter_context(tc.tile_pool(name="tp", bufs=4))
        wp = ctx.enter_context(tc.tile_pool(name="wp", bufs=4))
        oop = ctx.enter_context(tc.tile_pool(name="oop", bufs=4))

        # float activation bias requires a registered const AP; memset our
        # own [128,1] bias tile instead
        bt = consts.tile([p, 1], f32)
        nc.gpsimd.memset(bt[:], float(b))

        if combined_id is not None:
            nc.scalar.add_instruction(
                mybir.InstLoadActFuncSet(
                    name=nc.get_next_instruction_name(),
                    act_func_set_id=combined_id,
                )
            )

        for i in range(len(splits) - 1):
            lo, hi = splits[i], splits[i + 1]
            ft = hi - lo
            v = iop.tile([p, ft], bf16, tag="v")
            nc.sync.dma_start(out=v[:], in_=rin2d[:, lo:hi])
            t = tp.tile([p, ft], f16, tag="t")
            nc.scalar.activation(out=t[:], in_=v[:], func=A.Ln)
            w = wp.tile([p, ft], f16, tag="w")
            nc.scalar.activation(out=w[:], in_=t[:], func=A.Square,
                                 scale=float(a), bias=bt[:])
            o = oop.tile([p, ft], f16, tag="o")
            nc.vector.scalar_tensor_tensor(
                out=o[:], in0=w[:], scalar=float(s), in1=t[:],
                op0=ALU.add, op1=ALU.mult,
            )
            nc.sync.dma_start(out=out2d[:, lo:hi], in_=o[:])

    nc.finalize()
    return nc


def _fit_poly(c, t_min):
    """Weighted-lstsq (approx minimax) cubic fit of tanh(c*t) on
    [t_min, 0], factored as (Square(a*t+b)+s)*t = a^2 t^3 + 2ab t^2 +
    (b^2+s) t.  Returns (a, b, s, fit_maxerr, y_max) or None if the
    t^3 coefficient is not positive (the Square form needs a^2 = alpha)."""
    t = np.linspace(min(t_min, -1e-3), 0.0, 20001)
    y = np.tanh(c * t)
    A = np.stack([t ** 3, t ** 2, t], axis=1)
    w = np.ones_like(t)
    coef = np.zeros(3)
    for _ in range(25):
        coef, *_ = np.linalg.lstsq(A * w[:, None], y * w, rcond=None)
        err = A @ coef - y
        w = (w * (1 + np.abs(err) / max(np.abs(err).max(), 1e-30))) ** 0.8
        w /= w.max()
    al, be, ga = coef
    maxerr = float(np.abs(A @ coef - y).max())
    if not (al > 1e-30):
        return None
    a = float(np.sqrt(al))
    b = float(be / (2 * a))
    return a, b, float(ga - b * b), maxerr, float(np.abs(y).max())


def _build_fast(n_core=N_CORE, n_tiles=N_TILES):
    from concourse import bacc
    import concourse.tile as tile
    from concourse import mybir
    from contextlib import ExitStack

    f32 = mybir.dt.float32
    A = mybir.ActivationFunctionType
    ALU = mybir.AluOpType

    p = P
    f = n_core // p        # 2048 elements per partition
    ft = f // n_tiles

    nc = bacc.Bacc("TRN2", target_bir_lowering=False, debug=False)

    rad_d = nc.dram_tensor("radience", [n_core], f32, kind="ExternalInput")
    c_d = nc.dram_tensor("two_c", [1], f32, kind="ExternalInput")
    out_d = nc.dram_tensor("color", [n_core], f32, kind="ExternalOutput")
    rad2d = rad_d.ap().rearrange("(p f) -> p f", p=p)
    out2d = out_d.ap().rearrange("(p f) -> p f", p=p)

    # ln and exp never share a compiler-chosen table set; preload the one
    # set that holds both so no ACT table switch lands on the critical path
    combined_id = None
    try:
        from concourse.hw_specs import get_activation_tables
        names = list(get_activation_tables(nc.m.arch).keys())
        combined_id = names.index("natural_log_exp_and_others")
    except Exception:
        combined_id = None  # compiler inserts per-function loads instead

    with tile.TileContext(nc) as tc, ExitStack() as ctx:
        consts = ctx.enter_context(tc.tile_pool(name="consts", bufs=1))
        iop = ctx.enter_context(tc.tile_pool(name="iop", bufs=4))
        oop = ctx.enter_context(tc.tile_pool(name="oop", bufs=4))

        if combined_id is not None:
            nc.scalar.add_instruction(
                mybir.InstLoadActFuncSet(
                    name=nc.get_next_instruction_name(),
                    act_func_set_id=combined_id,
                )
            )

        cs = consts.tile([p, 1], f32)
        nc.sync.dma_start(out=cs[:], in_=c_d.ap().to_broadcast([p, 1]))

        for i in range(n_tiles):
            v = iop.tile([p, ft], f32, tag="v")
            nc.sync.dma_start(out=v[:], in_=rad2d[:, i * ft:(i + 1) * ft])
            nc.scalar.activation(out=v[:], in_=v[:], func=A.Ln)
            # v = exp(2c * ln r) = r^(2c); per-partition scale operand
            nc.scalar.activation(out=v[:], in_=v[:], func=A.Exp, scale=cs[:])
            w = oop.tile([p, ft], f32, tag="w")
            nc.vector.tensor_scalar(
                out=w[:], in0=v[:], scalar1=1.0, scalar2=None, op0=ALU.add
            )
            nc.vector.reciprocal_approx_fast(out=w[:], in_=w[:])
            # tanh = 1 - 2/(v+1)
            nc.vector.tensor_scalar(
                out=w[:], in0=w[:], scalar1=-2.0, scalar2=1.0,
                op0=ALU.mult, op1=ALU.add,
            )
            nc.sync.dma_start(out=out2d[:, i * ft:(i + 1) * ft], in_=w[:])

    nc.finalize()
    return nc


def _kernel_poly(radc, c_neg):
    """Device poly path; returns f32 [N_TOTAL] or None if the fit for
    this c is unusable."""
    global _BUILT_POLY
    from ml_dtypes import bfloat16

    t_min = float(np.log(max(float(radc.min()), EPS)))
    fit = _fit_poly(c_neg, t_min)
    if fit is None:
        return None
    a, b, s, maxerr, y_max = fit
    # the fit must hold well inside the 2e-2-of-absmax gate (fp16
    # rounding adds ~1e-3)
    if maxerr > 0.006 * max(y_max, 0.05):
        return None

    key = (round(a, 12), round(b, 12), round(s, 12))
    if _BUILT_POLY is None or _BUILT_POLY[0] != key:
        _BUILT_POLY = (key, _build_poly(a, b, s))
    nc = _BUILT_POLY[1]

    rin = radc.astype(bfloat16)
    in_maps = []
    for c in range(N_CORES):
        in_maps.append({
            "rin": np.ascontiguousarray(rin[c * N_CORE:(c + 1) * N_CORE]),
        })
    res = _run(nc, in_maps, list(range(N_CORES)))
    out = np.concatenate([res.results[c]["color16"] for c in range(N_CORES)])
    return out.astype(np.float32)


def _kernel_exp(radc, c_neg):
    global _BUILT_FAST
    if _BUILT_FAST is None:
        _BUILT_FAST = _build_fast()
    nc = _BUILT_FAST

    two_c = np.array([2.0 * c_neg], dtype=np.float32)
    in_maps = []
    for c in range(N_CORES):
        in_maps.append({
            "radience": np.ascontiguousarray(radc[c * N_CORE:(c + 1) * N_CORE]),
            "two_c": two_c,
        })
    res = _run(nc, in_maps, list(range(N_CORES)))
    out = np.concatenate([res.results[c]["color"] for c in range(N_CORES)])
    return np.asarray(out, dtype=np.float32)


def _kernel_fast(rad, inputs):
    W0 = np.asarray(inputs["W0"], dtype=np.float64).reshape(-1)
    W1 = np.asarray(inputs["W1"], dtype=np.float64)
    W2 = np.asarray(inputs["W2"], dtype=np.float64)
    W3 = np.asarray(inputs["W3"], dtype=np.float64).reshape(-1)
    c_neg = _collapse_scalar(W0, W1, W2, W3, -1.0)

    radc = np.maximum(rad, np.float32(EPS))  # safelog domain guard

    out = None
    if c_neg < 0:
        out = _kernel_poly(radc, c_neg)
    if out is None:
        out = _kernel_exp(radc, c_neg)
    out = out.reshape(N_TOTAL, 1)

    # pixels with r >= 1 (t >= 0) follow the positive-sign mask chain
    if float(rad.max()) >= 1.0:
        c_pos = _collapse_scalar(W0, W1, W2, W3, 1.0)
        idx = np.nonzero(rad >= 1.0)[0]
        out[idx, 0] = np.tanh(
            c_pos * np.log(np.maximum(rad[idx], EPS))
        ).astype(np.float32)
    return out


# ---------------------------------------------------------------------------
# fallback path: full per-pixel MLP (handles arbitrary biases)
# ---------------------------------------------------------------------------

def _build_bass(n_core=N_CORE, mm_dt_name="float16", finalize=True):
    from concourse import bacc
    import concourse.tile as tile
    from concourse import mybir
    from contextlib import ExitStack

    f32 = mybir.dt.float32
    mm_dt = getattr(mybir.dt, mm_dt_name)
    A = mybir.ActivationFunctionType
    ALU = mybir.AluOpType

    p = P
    f = n_core // p              # free dim per partition
    n_chunks = n_core // CH
    n_slabs = n_core // SLAB
    rows_per_slab = SLAB // f    # rad partition-rows gathered per slab
    assert n_chunks % 8 == 0 and rows_per_slab >= 1

    nc = bacc.Bacc("TRN2", target_bir_lowering=False, debug=False)

    rad_d = nc.dram_tensor("radience", [n_core], f32, kind="ExternalInput")
    out_d = nc.dram_tensor("color", [n_core], f32, kind="ExternalOutput")
    w0_d = nc.dram_tensor("W0", [1, 128], f32, kind="ExternalInput")
    b0_d = nc.dram_tensor("b0", [128], f32, kind="ExternalInput")
    w1_d = nc.dram_tensor("W1", [128, 128], f32, kind="ExternalInput")
    b1_d = nc.dram_tensor("b1", [128], f32, kind="ExternalInput")
    w2_d = nc.dram_tensor("W2", [128, 128], f32, kind="ExternalInput")
    b2_d = nc.dram_tensor("b2", [128], f32, kind="ExternalInput")
    w3_d = nc.dram_tensor("W3", [128, 32], f32, kind="ExternalInput")
    b3_d = nc.dram_tensor("b3", [1], f32, kind="ExternalInput")

    rad2d = rad_d.ap().rearrange("(p f) -> p f", p=p)
    out3d = out_d.ap().rearrange("(g r c) -> g r c", r=4, c=CH)

    with tile.TileContext(nc) as tc, ExitStack() as ctx:
        consts = ctx.enter_context(tc.tile_pool(name="consts", bufs=1))
        radp = ctx.enter_context(tc.tile_pool(name="radp", bufs=1))
        stgp = ctx.enter_context(tc.tile_pool(name="stgp", bufs=4))
        hp = ctx.enter_context(tc.tile_pool(name="hp", bufs=9))
        outp = ctx.enter_context(tc.tile_pool(name="outp", bufs=3))
        psp = ctx.enter_context(tc.tile_pool(name="psp", bufs=4, space="PSUM"))

        # --- constants ---
        # weights land as fp32 then are copy-converted to the matmul dtype
        # (fp32r consumers require producer-side rounding)
        w0f = consts.tile([1, 128], f32)
        nc.sync.dma_start(out=w0f[:], in_=w0_d.ap())
        w1f = consts.tile([128, 128], f32)
        nc.sync.dma_start(out=w1f[:], in_=w1_d.ap())
        w2f = consts.tile([128, 128], f32)
        nc.sync.dma_start(out=w2f[:], in_=w2_d.ap())
        # W3 arrives host-padded to 32 output columns (col 0 real, rest
        # zero) so each column-tiled layer-4 matmul initializes a full
        # 32-partition strip
        w3f = consts.tile([128, 32], f32)
        nc.sync.dma_start(out=w3f[:], in_=w3_d.ap())
        w0 = consts.tile([1, 128], mm_dt)
        nc.vector.tensor_copy(w0[:], w0f[:])
        # W0 replicated onto partitions {0,32,64,96}: layer-1 K=1 matmuls
        # run 4-concurrent on disjoint 32-row strips of the PE array
        w0q = consts.tile([97, 128], mm_dt)
        for _r in range(4):
            nc.sync.dma_start(out=w0q[32 * _r:32 * _r + 1, :], in_=w0[:])
        w1 = consts.tile([128, 128], mm_dt)
        nc.vector.tensor_copy(w1[:], w1f[:])
        w2 = consts.tile([128, 128], mm_dt)
        nc.vector.tensor_copy(w2[:], w2f[:])
        # layer-4 column-tiles, so it must use a 16-bit dtype
        w3 = consts.tile([128, 32], mm_dt)
        nc.vector.tensor_copy(w3[:], w3f[:])
        b0s = consts.tile([128, 1], f32)
        nc.sync.dma_start(out=b0s[:], in_=b0_d.ap().rearrange("(p f) -> p f", f=1))
        b1s = consts.tile([128, 1], f32)
        nc.sync.dma_start(out=b1s[:], in_=b1_d.ap().rearrange("(p f) -> p f", f=1))
        b2s = consts.tile([128, 1], f32)
        nc.sync.dma_start(out=b2s[:], in_=b2_d.ap().rearrange("(p f) -> p f", f=1))
        b3s = consts.tile([128, 1], f32)
        nc.sync.dma_start(out=b3s[:], in_=b3_d.ap().to_broadcast([128, 1]))

        # --- load pixels, safelog ---
        rad = radp.tile([p, f], f32)
        nc.sync.dma_start(out=rad[:], in_=rad2d)
        nc.vector.tensor_scalar(
            out=rad[:], in0=rad[:], scalar1=EPS, scalar2=None, op0=ALU.max
        )
        logr = radp.tile([p, f], mm_dt)
        nc.scalar.activation(out=logr[:], in_=rad[:], func=A.Ln)

        def relu_into(dst, src, bias, use_act):
            if use_act:
                nc.scalar.activation(out=dst, in_=src, func=A.Relu, bias=bias)
            else:
                nc.vector.tensor_scalar(
                    out=dst, in0=src, scalar1=bias, scalar2=0.0,
                    op0=ALU.add, op1=ALU.max,
                )

        prev = None  # software-pipelined layer 4 of slab s-1

        def emit_l4(pv):
            h3p, s_p = pv
            ps4 = psp.tile([128, 2 * CH], f32, tag="ps")
            for j in range(8):
                g, r = j // 4, j % 4
                srcp = h3p[j // 2][:, (j % 2) * CH:(j % 2 + 1) * CH]
                nc.tensor.matmul(
                    out=ps4[32 * r:32 * r + 32, g * CH:(g + 1) * CH],
                    lhsT=w3[:], rhs=srcp,
                    tile_position=(0, 32 * r),
                    skip_group_check=True,
                )
            ot = outp.tile([128, 2 * CH], f32, tag="ot")
            nc.scalar.activation(out=ot[:], in_=ps4[:], func=A.Tanh, bias=b3s[:])
            for g in range(2):
                nc.sync.dma_start(
                    out=out3d[2 * s_p + g, :, :],
                    in_=ot[0:128:32, g * CH:(g + 1) * CH],
                )

        for s in range(n_slabs):
            # gather this slab's log-pixels onto partitions {0,32,64,96}:
            # strip 32r gets chunk r (free 0:CH) and chunk 4+r (free CH:2CH)
            stg = stgp.tile([97, SLAB // 4], mm_dt, tag="stg")
            rs = s * rows_per_slab
            if rows_per_slab == 2:
                # each logr row covers 4 chunks -> one strided DMA per row
                for g in range(2):
                    nc.sync.dma_start(
                        out=stg[0:97:32, g * CH:(g + 1) * CH],
                        in_=logr[rs + g:rs + g + 1, :],
                    )
            else:
                for j in range(8):
                    px = s * SLAB + j * CH
                    row, col = px // f, px % f
                    nc.sync.dma_start(
                        out=stg[32 * (j % 4):32 * (j % 4) + 1,
                                (j // 4) * CH:(j // 4 + 1) * CH],
                        in_=logr[row:row + 1, col:col + CH],
                    )

            # ---- layers 1..3, layer-major so engine FIFOs never
            # head-of-line block: all matmuls of a layer back-to-back
            # (keeps the PE HAM-warm), relus split ACT/DVE per pair ----
            ps1s, h1s, ps2s, h2s, ps3s, h3 = [], [], [], [], [], []
            for q in range(4):
                ps1s.append(psp.tile([128, 2 * CH], f32, tag="ps", name=f"ps1_{s}_{q}"))
            for j in range(8):
                g, r = j // 4, j % 4
                nc.tensor.matmul(
                    out=ps1s[j // 2][:, (j % 2) * CH:(j % 2 + 1) * CH],
                    lhsT=w0q[32 * r:32 * r + 1, :],
                    rhs=stg[32 * r:32 * r + 1, g * CH:(g + 1) * CH],
                    tile_position=(32 * r, 0),
                    skip_group_check=True,
                )
            if prev is not None:
                emit_l4(prev)
            for q in range(4):
                h1 = hp.tile([128, 2 * CH], mm_dt, tag="h")
                relu_into(h1[:], ps1s[q][:], b0s[:], use_act=(q % 2 == 0))
                h1s.append(h1)
            for q in range(4):
                ps2 = psp.tile([128, 2 * CH], f32, tag="ps")
                nc.tensor.matmul(out=ps2[:, 0:CH], lhsT=w1[:],
                                 rhs=h1s[q][:, 0:CH])
                nc.tensor.matmul(out=ps2[:, CH:2 * CH], lhsT=w1[:],
                                 rhs=h1s[q][:, CH:2 * CH])
                ps2s.append(ps2)
            for q in range(4):
                h2 = hp.tile([128, 2 * CH], mm_dt, tag="h")
                relu_into(h2[:], ps2s[q][:], b1s[:], use_act=(q % 2 == 1))
                h2s.append(h2)
            for q in range(4):
                ps3 = psp.tile([128, 2 * CH], f32, tag="ps")
                nc.tensor.matmul(out=ps3[:, 0:CH], lhsT=w2[:],
                                 rhs=h2s[q][:, 0:CH])
                nc.tensor.matmul(out=ps3[:, CH:2 * CH], lhsT=w2[:],
                                 rhs=h2s[q][:, CH:2 * CH])
                ps3s.append(ps3)
            for q in range(4):
                h3q = hp.tile([128, 2 * CH], mm_dt, tag="h3")
                relu_into(h3q[:], ps3s[q][:], b2s[:], use_act=(q % 2 == 0))
                h3.append(h3q)

            prev = (h3, s)

        emit_l4(prev)

    if finalize:
        nc.finalize()
    return nc


def _kernel_mlp(rad, inputs):
    global _BUILT
    weights = {
        k: np.ascontiguousarray(np.asarray(inputs[k], dtype=np.float32))
        for k in ("W0", "b0", "W1", "b1", "W2", "b2", "W3", "b3")
    }
    weights["W3"] = np.ascontiguousarray(
        np.pad(weights["W3"].reshape(128, 1), ((0, 0), (0, 31)))
    )

    if _BUILT is None:
        _BUILT = _build_bass()
    nc = _BUILT

    in_maps = []
    for c in range(N_CORES):
        m = {"radience": np.ascontiguousarray(rad[c * N_CORE:(c + 1) * N_CORE])}
        m.update(weights)
        in_maps.append(m)

    res = _run(nc, in_maps, list(range(N_CORES)))
    out = np.concatenate([res.results[c]["color"] for c in range(N_CORES)])
    return out.reshape(N_TOTAL, 1)


def _run(nc, in_maps, core_ids, **kw):
    from concourse.bass_utils import run_bass_kernel_spmd
    return run_bass_kernel_spmd(nc, in_maps, core_ids, **kw)


def kernel(**inputs):
    rad = np.asarray(inputs["radience"], dtype=np.float32).reshape(-1)
    n = rad.shape[0]
    assert n == N_TOTAL, f"expected {N_TOTAL} pixels, got {n}"

    biases_zero = all(
        np.all(np.asarray(inputs[k]) == 0.0) for k in ("b0", "b1", "b2", "b3")
    )
    if biases_zero:
        return _kernel_fast(rad, inputs)
    return _kernel_mlp(rad, inputs)


if __name__ == "__main__":
    rng = np.random.default_rng(0)
    demo = {
        "radience": rng.random((N_TOTAL, 1), dtype=np.float32),
        "W0": rng.standard_normal((1, 128), dtype=np.float32) * 0.1,
        "b0": np.zeros(128, np.float32),
        "W1": rng.standard_normal((128, 128), dtype=np.float32) * 0.1,
        "b1": np.zeros(128, np.float32),
        "W2": rng.standard_normal((128, 128), dtype=np.float32) * 0.1,
        "b2": np.zeros(128, np.float32),
        "W3": rng.standard_normal((128, 1), dtype=np.float32) * 0.1,
        "b3": np.zeros(1, np.float32),
    }
    out = kernel(**demo)
    print("kernel out:", out.shape, out.dtype, out[:4, 0])
